# revision 2
# baseline (speedup 1.0000x reference)
"""Trainium2 Bass kernel v3 for nn_NestedNarx.

vs kernel2:
  - L1 via 4 concurrent tile_position matmuls [K=24, M=64] per strip
    (row groups 32b, col groups alternating), ~2x fewer PE columns.
  - L2 via 4 concurrent 64x64 quadrant matmuls (one per basin; q=1 pair
    row-swizzled, compensated in l3).
  - xp laid out chunk-major: [NCHUNK, NSTRIP, 4, 24, CH] so every DMA is
    one 24KB contiguous block.
  - All relu on DVE except N_ACT_RELU strips/chunk on ScalarE (balance).

Crossing engines (the wall): DVE relu FD=1024 ~1.3us/op, ScalarE tanh
FD=1024 ~1.15us/op, both PSUM fp32 -> SBUF bf16.
"""

import os
import sys

import numpy as np

for _p in ("/opt/trn_rl_repo",):
    if _p not in sys.path and os.path.isdir(_p):
        sys.path.insert(0, _p)

import concourse.bass as bass
import concourse.mybir as mybir
from concourse.tile import TileContext

F32 = mybir.dt.float32
BF16 = mybir.dt.bfloat16
AF = mybir.ActivationFunctionType
ALU = mybir.AluOpType

T = 4096
NG_ALL = 1024
NCORES = 8
G_CORE = NG_ALL // NCORES  # 128
NSTRIP = 32
HID = 64
CH = 512
NCHUNK = T // CH
ACT_RELU_STRIPS = {5, 16, 27}  # relu strips per chunk handled by ScalarE (spread)


def _split_multiwaits(nc):
    uid = [0]
    for fn in nc.m.functions:
        for bb in fn.blocks:
            new = []
            for inst in bb.instructions:
                si = inst.sync_info
                waits = list(si.on_wait) if si is not None and si.on_wait else []
                if len(waits) > 1:
                    for w in waits[:-1]:
                        uid[0] += 1
                        new.append(
                            mybir.InstNoOp(
                                name=f"{inst.name}-sw{uid[0]}",
                                engine=inst.engine,
                                bass_nofuse=True,
                                sync_info=mybir.SyncInfo(on_wait=[w], on_update=[]),
                            )
                        )
                    si.on_wait = waits[-1:]
                new.append(inst)
            bb.instructions = new


def build_nc():
    nc = bass.Bass()
    xp = nc.declare_dram_parameter("xp", [NCHUNK, 8, 128, 4 * CH], BF16, isOutput=False)
    l1 = nc.declare_dram_parameter("l1", [2, 128, 128], BF16, isOutput=False)
    l2 = nc.declare_dram_parameter("l2", [128, 128], BF16, isOutput=False)
    l3 = nc.declare_dram_parameter("l3", [8, 128, 16], BF16, isOutput=False)
    b1 = nc.declare_dram_parameter("b1", [128, 1], F32, isOutput=False)
    b2 = nc.declare_dram_parameter("b2", [128, 1], F32, isOutput=False)
    bo = nc.declare_dram_parameter("bo", [128, 1], F32, isOutput=False)
    y = nc.declare_dram_parameter("y", [G_CORE, T], F32, isOutput=True)

    with TileContext(nc) as tc:
        with (
            tc.tile_pool(name="const", bufs=1) as constp,
            tc.tile_pool(name="xs", bufs=3) as xsp,
            tc.tile_pool(name="hh", bufs=4) as hp,
            tc.tile_pool(name="aa", bufs=22) as ap_,
            tc.tile_pool(name="yout", bufs=2) as youtp,
            tc.tile_pool(name="psumh", bufs=2, space=bass.MemorySpace.PSUM) as pshp,
            tc.tile_pool(name="psuma", bufs=2, space=bass.MemorySpace.PSUM) as psap,
        ):
            # ---- constants ----
            l1t = {}
            for q in range(2):
                tl = constp.tile([128, 128], BF16, name=f"l1_{q}")
                nc.sync.dma_start(out=tl, in_=l1[q])
                l1t[q] = tl
            l2t = constp.tile([128, 128], BF16, name="l2t")
            nc.sync.dma_start(out=l2t, in_=l2[:])
            l3t = {}
            for p in range(8):
                tl = constp.tile([128, 16], BF16, name=f"l3_{p}")
                nc.sync.dma_start(out=tl, in_=l3[p])
                l3t[p] = tl
            b1t = constp.tile([128, 1], F32, name="b1t")
            nc.sync.dma_start(out=b1t, in_=b1[:])
            b2t = constp.tile([128, 1], F32, name="b2t")
            nc.sync.dma_start(out=b2t, in_=b2[:])
            bot = constp.tile([128, 1], F32, name="bot")
            nc.sync.dma_start(out=bot, in_=bo[:])
            # warm the activation table (tanh set includes relu)
            warmt = constp.tile([128, 1], BF16, name="warmt")
            nc.scalar.activation(warmt, b1t, AF.Tanh, bias=b2t)

            Atiles = {}

            def do_l3_round(c, R):
                # blocks 4R..4R+3 of chunk c (strips 16R..16R+15)
                tc0 = c * CH
                psY = psap.tile([128, 2 * CH], F32, name="psY", tag="psA")
                for p in range(8):
                    for j in range(4):
                        J = 4 * R + j
                        s = 4 * J + p // 2
                        q = p % 2
                        nc.tensor.matmul(
                            psY[32 * j : 32 * j + 16, 0:CH],
                            l3t[p],
                            Atiles[(c, s)][:, CH * q : CH * q + CH],
                            start=(p == 0),
                            stop=(p == 7),
                            tile_position=(0, 32 * j),
                        )
                ysb = youtp.tile([112, CH], F32, name="ysb", tag="ysb")
                nc.vector.tensor_scalar_add(ysb, psY[0:112, 0:CH], bot[0:112, :])
                for j in range(4):
                    J = 4 * R + j
                    nc.sync.dma_start(
                        out=y[16 * J : 16 * J + 16, tc0 : tc0 + CH],
                        in_=ysb[32 * j : 32 * j + 16, :],
                    )
                for j in range(4):
                    for ss in range(4 * (4 * R + j), 4 * (4 * R + j) + 4):
                        Atiles.pop((c, ss), None)

            for ck in range(NCHUNK):
                t0 = ck * CH

                for s in range(NSTRIP):
                    g, u = s // 4, s % 4
                    if u == 0:
                        xs4 = xsp.tile([128, 4 * CH], BF16, name="xs", tag="xs")
                        nc.sync.dma_start(out=xs4, in_=xp[ck, g])
                    xs = xs4[:, CH * u : CH * u + CH]

                    # L1: 2 full-array block-diag matmuls (q = basin pair)
                    psH = pshp.tile([128, 2 * CH], F32, name="psH", tag="psH")
                    for q in range(2):
                        nc.tensor.matmul(
                            psH[:, CH * q : CH * q + CH],
                            l1t[q],
                            xs,
                            start=True,
                            stop=True,
                        )
                    H = hp.tile([128, 2 * CH], BF16, name="H", tag="H")
                    if s in ACT_RELU_STRIPS:
                        nc.scalar.activation(H, psH, AF.Relu, bias=b1t)
                    else:
                        nc.vector.tensor_scalar(H, psH, b1t, 0.0, ALU.add, ALU.max)

                    # L2: 2 full-array block-diag matmuls
                    psA = psap.tile([128, 2 * CH], F32, name="psA", tag="psA")
                    for q in range(2):
                        nc.tensor.matmul(
                            psA[:, CH * q : CH * q + CH],
                            l2t,
                            H[:, CH * q : CH * q + CH],
                            start=True,
                            stop=True,
                        )
                    A = ap_.tile([128, 2 * CH], BF16, name="A", tag="A")
                    nc.scalar.activation(A, psA, AF.Tanh, bias=b2t)
                    Atiles[(ck, s)] = A

                    if s == 17:
                        do_l3_round(ck, 0)
                    if s == 1 and ck > 0:
                        do_l3_round(ck - 1, 1)
            do_l3_round(NCHUNK - 1, 1)
    _split_multiwaits(nc)
    return nc


def prep_weights(W_in, b_in, W_ih, b_ih, b_hh, W_out, b_out):
    import ml_dtypes

    W_in = np.asarray(W_in, np.float32)
    A = np.zeros((3, HID, 8), np.float32)
    A[0, :, 0:7] = W_in[:, 0:7] + W_in[:, 21:28]
    A[0, :, 7] = W_in[:, 28] + W_in[:, 31]
    A[1, :, 0:7] = W_in[:, 14:21]
    A[1, :, 7] = W_in[:, 30]
    A[2, :, 0:7] = W_in[:, 7:14]
    A[2, :, 7] = W_in[:, 29]
    # per-basin L1 weight [24, 64]
    l1b = np.zeros((24, 64), np.float32)
    for d in range(3):
        l1b[8 * d : 8 * d + 8, :] = A[d].T
    # block-diag over padded 32-row basin slots: pair q -> basins (2q, 2q+1)
    l1 = np.zeros((2, 128, 128), np.float32)
    for q in range(2):
        for c in range(2):
            b = 2 * q + c
            l1[q, 32 * b : 32 * b + 24, 64 * c : 64 * c + 64] = l1b

    l2 = np.zeros((128, 128), np.float32)
    l2[0:64, 0:64] = np.asarray(W_ih, np.float32).T
    l2[64:128, 64:128] = np.asarray(W_ih, np.float32).T

    W_out = np.asarray(W_out, np.float32)  # [1, 64]
    # pair p (s_off=p//2, q=p%2): rows 0:64 -> basin m0, rows 64:128 -> m0+1
    l3 = np.zeros((8, 128, 16), np.float32)
    for p in range(8):
        m0 = 4 * (p // 2) + 2 * (p % 2)
        l3[p, 0:64, m0] = W_out[0]
        l3[p, 64:128, m0 + 1] = W_out[0]

    b1 = np.concatenate([b_in, b_in]).astype(np.float32).reshape(128, 1)
    bb = np.asarray(b_ih, np.float32) + np.asarray(b_hh, np.float32)
    b2 = np.concatenate([bb, bb]).astype(np.float32).reshape(128, 1)
    bo = np.full((128, 1), np.asarray(b_out, np.float32).reshape(-1)[0], np.float32)
    bf = ml_dtypes.bfloat16
    return l1.astype(bf), l2.astype(bf), l3.astype(bf), b1, b2, bo


def prep_x_core(x, core):
    """x [T, 1024, 8] -> xp [NCHUNK, 8, 128, 4*CH] bf16 delay-stacked.

    xp[ck, g, 32*b + f, CH*u + t] = feat f of basin-slot b of strip 4g+u
    (rows 24..31 of each 32-row group are zero padding)."""
    import ml_dtypes

    xc = np.asarray(x[:, core * G_CORE : (core + 1) * G_CORE, :], np.float32)
    xcm = np.ascontiguousarray(xc.transpose(1, 2, 0))  # [g, c, t]
    st = np.zeros((G_CORE, 32, T), np.float32)  # [basin, padded feat, t]
    for d in (1, 2, 3):
        st[:, 8 * (d - 1) : 8 * d, d:] = xcm[:, :, : T - d]
    # basin = 4*(4*g + u) + b ; want [ck, g, b, f, u, t]
    st = st.reshape(8, 4, 4, 32, NCHUNK, CH)  # [g, u, b, f, ck, t]
    out = st.transpose(4, 0, 2, 3, 1, 5)  # [ck, g, b, f, u, t]
    return np.ascontiguousarray(out).reshape(
        NCHUNK, 8, 128, 4 * CH
    ).astype(ml_dtypes.bfloat16)


_NC_CACHE = {}


def _get_nc():
    if "nc" not in _NC_CACHE:
        _NC_CACHE["nc"] = build_nc()
    return _NC_CACHE["nc"]


def kernel(x, W_in, b_in, W_ih, b_ih, W_hh, b_hh, W_out, b_out, _trace=False):
    from concourse.bass_utils import run_bass_kernel_spmd

    x = np.asarray(x, np.float32)
    l1, l2, l3, b1, b2, bo = prep_weights(W_in, b_in, W_ih, b_ih, b_hh, W_out, b_out)
    in_maps = []
    for core in range(NCORES):
        in_maps.append(
            {
                "xp": prep_x_core(x, core),
                "l1": l1,
                "l2": l2,
                "l3": l3,
                "b1": b1,
                "b2": b2,
                "bo": bo,
            }
        )
    nc = _get_nc()
    res = run_bass_kernel_spmd(nc, in_maps, list(range(NCORES)), trace=_trace)
    _NC_CACHE["last_result"] = res

    out = np.empty((T, NG_ALL, 1), np.float32)
    out[:3, :, 0] = x[:3, :, 7]
    for core in range(NCORES):
        yc = res.results[core]["y"]  # [128, T]
        out[3:, core * G_CORE : (core + 1) * G_CORE, 0] = yc[:, 3:].T
    return out


# revision 4
# speedup vs baseline: 1.0067x; 1.0067x over previous
"""Trainium2 Bass kernel for nn_NestedNarx (batched 24->64->64->1 MLP
over 1024 basins x 4096 timesteps; pure data-parallel over 8 cores).

Design (per core, 128 basins, 8 time chunks of 512):
  - bf16 weights/activations on the PE; fp32 PSUM.
  - L1/L2 as full-array block-diagonal matmuls (2 basins per 512-col
    matmul) -- full-array matmuls keep the PE HAM clock warm at 2.4GHz
    (tile_position-only kernels never warm up and run at 1.2GHz).
  - L3 via 4 concurrent tile_position col-tiles [K=128, M=16] with
    8-matmul accumulation, deferred until its tanh inputs are already
    drained so the PE never idles >3.4us (HAM re-throttle window).
  - The wall is the PSUM->SBUF activation crossing (64 relu + 64 tanh
    per position = 67M elems/core): DVE tensor_scalar(add,max) FD=1024
    ~1.25us/op and ScalarE tanh FD=1024 ~1.11us/op; 3 relu strips per
    chunk go to ScalarE to balance the two engines (both ~77% busy).
  - xp is delay-stacked host-side, chunk-major, padded to 32-row basin
    slots so each 4-strip group loads with one 512KB contiguous DMA
    (many small DMAs bottleneck the sync queue at ~0.7us per issue).
  - psY steals psa-pool buffers between crossings; y bias-add on DVE.
"""

import os
import sys

import numpy as np

for _p in ("/opt/trn_rl_repo",):
    if _p not in sys.path and os.path.isdir(_p):
        sys.path.insert(0, _p)

import concourse.bass as bass
import concourse.mybir as mybir
from concourse.tile import TileContext

F32 = mybir.dt.float32
BF16 = mybir.dt.bfloat16
AF = mybir.ActivationFunctionType
ALU = mybir.AluOpType

T = 4096
NG_ALL = 1024
NCORES = 8
G_CORE = NG_ALL // NCORES  # 128
NSTRIP = 32
HID = 64
CH = 512
NCHUNK = T // CH
ACT_RELU_STRIPS = {5, 16, 27}  # relu strips per chunk handled by ScalarE (spread)


def _split_multiwaits(nc):
    uid = [0]
    for fn in nc.m.functions:
        for bb in fn.blocks:
            new = []
            for inst in bb.instructions:
                si = inst.sync_info
                waits = list(si.on_wait) if si is not None and si.on_wait else []
                if len(waits) > 1:
                    for w in waits[:-1]:
                        uid[0] += 1
                        new.append(
                            mybir.InstNoOp(
                                name=f"{inst.name}-sw{uid[0]}",
                                engine=inst.engine,
                                bass_nofuse=True,
                                sync_info=mybir.SyncInfo(on_wait=[w], on_update=[]),
                            )
                        )
                    si.on_wait = waits[-1:]
                new.append(inst)
            bb.instructions = new


def build_nc():
    nc = bass.Bass()
    xp = nc.declare_dram_parameter("xp", [NCHUNK, 8, 128, 4 * CH], BF16, isOutput=False)
    l1 = nc.declare_dram_parameter("l1", [2, 128, 128], BF16, isOutput=False)
    l2 = nc.declare_dram_parameter("l2", [128, 128], BF16, isOutput=False)
    l3 = nc.declare_dram_parameter("l3", [8, 128, 16], BF16, isOutput=False)
    b1 = nc.declare_dram_parameter("b1", [128, 1], F32, isOutput=False)
    b2 = nc.declare_dram_parameter("b2", [128, 1], F32, isOutput=False)
    bo = nc.declare_dram_parameter("bo", [128, 1], F32, isOutput=False)
    y = nc.declare_dram_parameter("y", [G_CORE, T], F32, isOutput=True)

    with TileContext(nc) as tc:
        with (
            tc.tile_pool(name="const", bufs=1) as constp,
            tc.tile_pool(name="xs", bufs=4) as xsp,
            tc.tile_pool(name="hh", bufs=4) as hp,
            tc.tile_pool(name="aa", bufs=22) as ap_,
            tc.tile_pool(name="yout", bufs=2) as youtp,
            tc.tile_pool(name="psumh", bufs=2, space=bass.MemorySpace.PSUM) as pshp,
            tc.tile_pool(name="psuma", bufs=2, space=bass.MemorySpace.PSUM) as psap,
        ):
            # ---- constants ----
            l1t = {}
            for q in range(2):
                tl = constp.tile([128, 128], BF16, name=f"l1_{q}")
                nc.sync.dma_start(out=tl, in_=l1[q])
                l1t[q] = tl
            l2t = constp.tile([128, 128], BF16, name="l2t")
            nc.sync.dma_start(out=l2t, in_=l2[:])
            l3t = {}
            for p in range(8):
                tl = constp.tile([128, 16], BF16, name=f"l3_{p}")
                nc.sync.dma_start(out=tl, in_=l3[p])
                l3t[p] = tl
            b1t = constp.tile([128, 1], F32, name="b1t")
            nc.sync.dma_start(out=b1t, in_=b1[:])
            b2t = constp.tile([128, 1], F32, name="b2t")
            nc.sync.dma_start(out=b2t, in_=b2[:])
            bot = constp.tile([128, 1], F32, name="bot")
            nc.sync.dma_start(out=bot, in_=bo[:])
            # warm the activation table (tanh set includes relu)
            warmt = constp.tile([128, 1], BF16, name="warmt")
            nc.scalar.activation(warmt, b1t, AF.Tanh, bias=b2t)

            Atiles = {}

            def do_l3_round(c, R):
                # blocks 4R..4R+3 of chunk c (strips 16R..16R+15)
                tc0 = c * CH
                psY = psap.tile([128, 2 * CH], F32, name="psY", tag="psA")
                for p in range(8):
                    for j in range(4):
                        J = 4 * R + j
                        s = 4 * J + p // 2
                        q = p % 2
                        nc.tensor.matmul(
                            psY[32 * j : 32 * j + 16, 0:CH],
                            l3t[p],
                            Atiles[(c, s)][:, CH * q : CH * q + CH],
                            start=(p == 0),
                            stop=(p == 7),
                            tile_position=(0, 32 * j),
                        )
                ysb = youtp.tile([112, CH], F32, name="ysb", tag="ysb")
                nc.vector.tensor_scalar_add(ysb, psY[0:112, 0:CH], bot[0:112, :])
                for j in range(4):
                    J = 4 * R + j
                    nc.sync.dma_start(
                        out=y[16 * J : 16 * J + 16, tc0 : tc0 + CH],
                        in_=ysb[32 * j : 32 * j + 16, :],
                    )
                for j in range(4):
                    for ss in range(4 * (4 * R + j), 4 * (4 * R + j) + 4):
                        Atiles.pop((c, ss), None)

            for ck in range(NCHUNK):
                t0 = ck * CH

                for s in range(NSTRIP):
                    g, u = s // 4, s % 4
                    if u == 0:
                        xs4 = xsp.tile([128, 4 * CH], BF16, name="xs", tag="xs")
                        nc.sync.dma_start(out=xs4, in_=xp[ck, g])
                    xs = xs4[:, CH * u : CH * u + CH]

                    # L1: 2 full-array block-diag matmuls (q = basin pair)
                    psH = pshp.tile([128, 2 * CH], F32, name="psH", tag="psH")
                    for q in range(2):
                        nc.tensor.matmul(
                            psH[:, CH * q : CH * q + CH],
                            l1t[q],
                            xs,
                            start=True,
                            stop=True,
                        )
                    H = hp.tile([128, 2 * CH], BF16, name="H", tag="H")
                    if s in ACT_RELU_STRIPS:
                        nc.scalar.activation(H, psH, AF.Relu, bias=b1t)
                    else:
                        nc.vector.tensor_scalar(H, psH, b1t, 0.0, ALU.add, ALU.max)

                    # L2: 2 full-array block-diag matmuls
                    psA = psap.tile([128, 2 * CH], F32, name="psA", tag="psA")
                    for q in range(2):
                        nc.tensor.matmul(
                            psA[:, CH * q : CH * q + CH],
                            l2t,
                            H[:, CH * q : CH * q + CH],
                            start=True,
                            stop=True,
                        )
                    A = ap_.tile([128, 2 * CH], BF16, name="A", tag="A")
                    nc.scalar.activation(A, psA, AF.Tanh, bias=b2t)
                    Atiles[(ck, s)] = A

                    if s == 17:
                        do_l3_round(ck, 0)
                    if s == 1 and ck > 0:
                        do_l3_round(ck - 1, 1)
            do_l3_round(NCHUNK - 1, 1)
    _split_multiwaits(nc)
    return nc


def prep_weights(W_in, b_in, W_ih, b_ih, b_hh, W_out, b_out):
    import ml_dtypes

    W_in = np.asarray(W_in, np.float32)
    A = np.zeros((3, HID, 8), np.float32)
    A[0, :, 0:7] = W_in[:, 0:7] + W_in[:, 21:28]
    A[0, :, 7] = W_in[:, 28] + W_in[:, 31]
    A[1, :, 0:7] = W_in[:, 14:21]
    A[1, :, 7] = W_in[:, 30]
    A[2, :, 0:7] = W_in[:, 7:14]
    A[2, :, 7] = W_in[:, 29]
    # per-basin L1 weight [24, 64]
    l1b = np.zeros((24, 64), np.float32)
    for d in range(3):
        l1b[8 * d : 8 * d + 8, :] = A[d].T
    # block-diag over padded 32-row basin slots: pair q -> basins (2q, 2q+1)
    l1 = np.zeros((2, 128, 128), np.float32)
    for q in range(2):
        for c in range(2):
            b = 2 * q + c
            l1[q, 32 * b : 32 * b + 24, 64 * c : 64 * c + 64] = l1b

    l2 = np.zeros((128, 128), np.float32)
    l2[0:64, 0:64] = np.asarray(W_ih, np.float32).T
    l2[64:128, 64:128] = np.asarray(W_ih, np.float32).T

    W_out = np.asarray(W_out, np.float32)  # [1, 64]
    # pair p (s_off=p//2, q=p%2): rows 0:64 -> basin m0, rows 64:128 -> m0+1
    l3 = np.zeros((8, 128, 16), np.float32)
    for p in range(8):
        m0 = 4 * (p // 2) + 2 * (p % 2)
        l3[p, 0:64, m0] = W_out[0]
        l3[p, 64:128, m0 + 1] = W_out[0]

    b1 = np.concatenate([b_in, b_in]).astype(np.float32).reshape(128, 1)
    bb = np.asarray(b_ih, np.float32) + np.asarray(b_hh, np.float32)
    b2 = np.concatenate([bb, bb]).astype(np.float32).reshape(128, 1)
    bo = np.full((128, 1), np.asarray(b_out, np.float32).reshape(-1)[0], np.float32)
    bf = ml_dtypes.bfloat16
    return l1.astype(bf), l2.astype(bf), l3.astype(bf), b1, b2, bo


def prep_x_core(x, core):
    """x [T, 1024, 8] -> xp [NCHUNK, 8, 128, 4*CH] bf16 delay-stacked.

    xp[ck, g, 32*b + f, CH*u + t] = feat f of basin-slot b of strip 4g+u
    (rows 24..31 of each 32-row group are zero padding)."""
    import ml_dtypes

    xc = np.asarray(x[:, core * G_CORE : (core + 1) * G_CORE, :], np.float32)
    xcm = np.ascontiguousarray(xc.transpose(1, 2, 0))  # [g, c, t]
    st = np.zeros((G_CORE, 32, T), np.float32)  # [basin, padded feat, t]
    for d in (1, 2, 3):
        st[:, 8 * (d - 1) : 8 * d, d:] = xcm[:, :, : T - d]
    # basin = 4*(4*g + u) + b ; want [ck, g, b, f, u, t]
    st = st.reshape(8, 4, 4, 32, NCHUNK, CH)  # [g, u, b, f, ck, t]
    out = st.transpose(4, 0, 2, 3, 1, 5)  # [ck, g, b, f, u, t]
    return np.ascontiguousarray(out).reshape(
        NCHUNK, 8, 128, 4 * CH
    ).astype(ml_dtypes.bfloat16)


_NC_CACHE = {}


def _get_nc():
    if "nc" not in _NC_CACHE:
        _NC_CACHE["nc"] = build_nc()
    return _NC_CACHE["nc"]


def kernel(x, W_in, b_in, W_ih, b_ih, W_hh, b_hh, W_out, b_out, _trace=False):
    from concourse.bass_utils import run_bass_kernel_spmd

    x = np.asarray(x, np.float32)
    l1, l2, l3, b1, b2, bo = prep_weights(W_in, b_in, W_ih, b_ih, b_hh, W_out, b_out)
    in_maps = []
    for core in range(NCORES):
        in_maps.append(
            {
                "xp": prep_x_core(x, core),
                "l1": l1,
                "l2": l2,
                "l3": l3,
                "b1": b1,
                "b2": b2,
                "bo": bo,
            }
        )
    nc = _get_nc()
    res = run_bass_kernel_spmd(nc, in_maps, list(range(NCORES)), trace=_trace)
    _NC_CACHE["last_result"] = res

    out = np.empty((T, NG_ALL, 1), np.float32)
    out[:3, :, 0] = x[:3, :, 7]
    for core in range(NCORES):
        yc = res.results[core]["y"]  # [128, T]
        out[3:, core * G_CORE : (core + 1) * G_CORE, 0] = yc[:, 3:].T
    return out
